# revision 7
# baseline (speedup 1.0000x reference)
"""Trainium2 Bass kernel for nn_AbstractReLU (interval-bound ReLU relaxation).

Computation (per column j of x[1026, N]):
  row 0 = center, rows 1..1024 = symbol coefs, row 1025 = noise.
  abs_sum = sum_{r>=1} |x[r,j]|;  lo = x0-abs_sum; hi = x0+abs_sum
  crossing (lo<0<hi):  mult = hi/(hi-lo), addend = 0.5*hi*(1-mult)
  positive (lo>=0,hi>0): mult = 1, addend = 0;  else mult = 0, addend = 0
  out row 0   = mult*x0 + addend
  out rows mid= mult*x
  out row 1025= mult*x_last + addend        (mult >= 0 so |mult| == mult)
  abs_sum2 = mult*(abs_sum - |x_last|) + |out_last|
  x_min_out = out0 - abs_sum2 ; x_max_out = out0 + abs_sum2
  x_true_out = relu(x_true)

Sharding: feature dim (axis 1 of x) split evenly across 8 cores; all
reductions are over the row axis which stays local -> no collectives.

Kernel layout per core (shard width NC):
  - rows on partitions: 9 row-tiles (8x128 + 1x2); column groups of G cols.
  - |x| on ScalarE, partition-reduce via ones-vector matmuls on TensorE
    accumulating into PSUM [1,512] banks.
  - per-column scalar chain on [32, NC/128] slab slices on VectorE.
  - mult broadcast to 128 partitions via K=1 matmul; 9 in-place DVE
    multiplies; stores via gpsimd DMA.
"""
import sys

for _p in ("/opt/trn_rl_repo",):
    if _p not in sys.path:
        sys.path.insert(0, _p)

import numpy as np
from contextlib import ExitStack

import concourse.bass as bass
import concourse.bacc as bacc
import concourse.tile as tile
from concourse import mybir

ROWS = 1026
N_FULL = 65536
N_CORES = 8

OP = mybir.AluOpType
DT = mybir.dt.float32
AF = mybir.ActivationFunctionType


def row_tiles():
    """(row_start, n_rows) for the 9 row tiles: 8x128 + 1x2."""
    ts = [(k * 128, 128) for k in range(8)]
    ts.append((1024, 2))
    return ts


def build_kernel(nc: bass.Bass, n_cols: int, group: int):
    """Emit the kernel program for one core's shard of width n_cols."""
    assert n_cols % 128 == 0 and n_cols % group == 0 and group % 512 == 0
    W = n_cols // 128          # slab free width (cols per partition)
    NG = n_cols // group       # number of column groups
    PPG = 128 // NG            # slab partitions per group
    assert PPG * W == group and PPG % 32 == 0 or NG == 1
    SUB = group // 512         # 512-col sub-blocks per group (PSUM bank size)
    PPS = 512 // W             # slab partitions per sub-block

    x_in = nc.dram_tensor("x", [ROWS, n_cols], DT, kind="ExternalInput").ap()
    xtrue_in = nc.dram_tensor("x_true", [n_cols], DT, kind="ExternalInput").ap()
    x_out = nc.dram_tensor("out_x", [ROWS, n_cols], DT, kind="ExternalOutput").ap()
    min_out = nc.dram_tensor("out_min", [n_cols], DT, kind="ExternalOutput").ap()
    max_out = nc.dram_tensor("out_max", [n_cols], DT, kind="ExternalOutput").ap()
    true_out = nc.dram_tensor("out_true", [n_cols], DT, kind="ExternalOutput").ap()

    tiles = row_tiles()

    with tile.TileContext(nc) as tc, ExitStack() as ctx:
        xtp = ctx.enter_context(tc.tile_pool(name="xt", bufs=2))
        abp = ctx.enter_context(tc.tile_pool(name="ab", bufs=3))
        srp = ctx.enter_context(tc.tile_pool(name="sr", bufs=4))
        mrp = ctx.enter_context(tc.tile_pool(name="mr", bufs=4))
        slb = ctx.enter_context(tc.tile_pool(name="slb", bufs=1))
        onp = ctx.enter_context(tc.tile_pool(name="on", bufs=1))
        psa = ctx.enter_context(tc.tile_pool(name="psa", bufs=4, space="PSUM"))
        psb = ctx.enter_context(tc.tile_pool(name="psb", bufs=1, space="PSUM"))

        V, A = nc.vector, nc.scalar

        ones_col = onp.tile([128, 1], DT)     # lhsT for partition reduce
        ones_row = onp.tile([1, 128], DT)     # lhsT for broadcast
        V.memset(ones_col[:], 1.0)
        V.memset(ones_row[:], 1.0)

        # Persistent [128, W] slabs: column j <-> (partition j//W, free j%W).
        def slab(name):
            return slb.tile([128, W], DT, name=name)

        sall = slab("sall")   # sum |x| over ALL rows
        x0s = slab("x0s")
        xls = slab("xls")
        s1 = slab("s1")
        lo = slab("lo")
        hi = slab("hi")
        hip = slab("hip")
        maskp = slab("maskp")
        den = slab("den")
        rec = slab("rec")
        coef = slab("coef")
        mult = slab("mult")
        adde = slab("adde")
        new0 = slab("new0")
        newl = slab("newl")
        t1 = slab("t1")
        t2 = slab("t2")
        s2v = slab("s2v")
        minv = slab("minv")
        maxv = slab("maxv")

        for g in range(NG):
            c0 = g * group
            cs = slice(c0, c0 + group)
            # ---- load the 9 row tiles of this column group
            xt = []
            for k, (r0, nr) in enumerate(tiles):
                t = xtp.tile([nr, group], DT, tag=f"xt{k}", name=f"xt{k}_g{g}")
                nc.sync.dma_start(out=t[:], in_=x_in[r0:r0 + nr, cs])
                xt.append(t)

            # ---- |x| and partition-reduce into PSUM accs (one per 512 cols)
            accs = [psa.tile([1, 512], DT, tag="acc", name=f"acc{s}_g{g}")
                    for s in range(SUB)]
            for k, (r0, nr) in enumerate(tiles):
                ab = abp.tile([nr, group], DT, tag="ab", name=f"ab{k}_g{g}")
                A.activation(ab[:], xt[k][:], AF.Abs)
                for s in range(SUB):
                    nc.tensor.matmul(
                        accs[s][:],
                        ones_col[0:nr, :],
                        ab[:, s * 512:(s + 1) * 512],
                        start=(k == 0),
                        stop=(k == len(tiles) - 1),
                    )

            # ---- move per-column sums into slab layout
            for s in range(SUB):
                srow = srp.tile([1, 512], DT, tag="sr", name=f"sr{s}_g{g}")
                A.activation(srow[:], accs[s][:], AF.Copy)
                p0 = g * PPG + s * PPS
                nc.gpsimd.dma_start(out=sall[p0:p0 + PPS, :], in_=srow[:])

            # row 0 and row 1025 into slab layout
            p0 = g * PPG
            gs = (slice(p0, p0 + PPG), slice(0, W))
            nc.gpsimd.dma_start(out=x0s[gs], in_=xt[0][0:1, :])
            nc.gpsimd.dma_start(out=xls[gs], in_=xt[8][1:2, :])

            # ---- per-column scalar chain on [PPG, W] slices
            A.activation(t1[gs], x0s[gs], AF.Abs)                      # |x0|
            V.tensor_sub(s1[gs], sall[gs], t1[gs])                     # sum rows>=1
            V.tensor_sub(lo[gs], x0s[gs], s1[gs])
            V.tensor_add(hi[gs], x0s[gs], s1[gs])
            V.tensor_scalar(hip[gs], hi[gs], 0.0, None, OP.is_gt)
            V.tensor_scalar(t2[gs], lo[gs], 0.0, None, OP.is_lt)
            V.tensor_mul(maskp[gs], hip[gs], t2[gs])
            V.tensor_sub(den[gs], hi[gs], lo[gs])
            V.tensor_scalar(den[gs], den[gs], 1e-20, None, OP.max)
            V.reciprocal(rec[gs], den[gs])
            V.tensor_mul(coef[gs], hi[gs], rec[gs])
            V.tensor_scalar(coef[gs], coef[gs], 1.0, 0.0, OP.min, OP.max)
            V.tensor_mul(t1[gs], maskp[gs], coef[gs])
            V.tensor_sub(t2[gs], hip[gs], maskp[gs])
            V.tensor_add(mult[gs], t1[gs], t2[gs])
            V.tensor_scalar(t1[gs], coef[gs], -0.5, 0.5, OP.mult, OP.add)
            V.tensor_mul(t1[gs], t1[gs], hi[gs])                       # bias
            V.tensor_mul(adde[gs], maskp[gs], t1[gs])                  # addend
            V.tensor_mul(t1[gs], mult[gs], x0s[gs])
            V.tensor_add(new0[gs], t1[gs], adde[gs])
            V.tensor_mul(t1[gs], mult[gs], xls[gs])
            V.tensor_add(newl[gs], t1[gs], adde[gs])
            A.activation(t1[gs], xls[gs], AF.Abs)                      # |xl|
            V.tensor_sub(t2[gs], s1[gs], t1[gs])                       # mid sum
            V.tensor_mul(t2[gs], mult[gs], t2[gs])
            A.activation(t1[gs], newl[gs], AF.Abs)                     # |newl|
            V.tensor_add(s2v[gs], t2[gs], t1[gs])
            V.tensor_sub(minv[gs], new0[gs], s2v[gs])
            V.tensor_add(maxv[gs], new0[gs], s2v[gs])

            # ---- broadcast mult across 128 partitions via K=1 matmul
            bc = psb.tile([128, group], DT, tag="bc", name=f"bc_g{g}")
            for s in range(SUB):
                mrow = mrp.tile([1, 512], DT, tag="mr", name=f"mr{s}_g{g}")
                p0 = g * PPG + s * PPS
                nc.gpsimd.dma_start(out=mrow[:], in_=mult[p0:p0 + PPS, :])
                nc.tensor.matmul(
                    bc[:, s * 512:(s + 1) * 512],
                    ones_row[:],
                    mrow[:],
                    start=True,
                    stop=True,
                )

            # ---- scale rows in place and store
            for k, (r0, nr) in enumerate(tiles):
                V.tensor_mul(xt[k][:], xt[k][:], bc[0:nr, :])
            nc.gpsimd.dma_start(out=x_out[1:128, cs], in_=xt[0][1:128, :])
            for k in range(1, 8):
                r0 = k * 128
                nc.gpsimd.dma_start(out=x_out[r0:r0 + 128, cs], in_=xt[k][:])
            nc.gpsimd.dma_start(out=x_out[1024:1025, cs], in_=xt[8][0:1, :])

        # ---- x_true relu
        xt_t = onp.tile([128, W], DT, name="xt_t")
        nc.sync.dma_start(out=xt_t[:], in_=xtrue_in[:])
        V.tensor_scalar(xt_t[:], xt_t[:], 0.0, None, OP.max)
        nc.gpsimd.dma_start(out=true_out[:], in_=xt_t[:])

        # ---- final row/vector outputs from slabs
        nc.gpsimd.dma_start(out=x_out[0:1, :], in_=new0[:])
        nc.gpsimd.dma_start(out=x_out[1025:1026, :], in_=newl[:])
        nc.gpsimd.dma_start(out=min_out[:], in_=minv[:])
        nc.gpsimd.dma_start(out=max_out[:], in_=maxv[:])

    return nc


_CACHED = {}


def _get_program(n_cols: int, group: int):
    key = (n_cols, group)
    if key not in _CACHED:
        nc = bacc.Bacc("TRN2", target_bir_lowering=False, debug=False,
                       num_devices=N_CORES)
        build_kernel(nc, n_cols, group)
        nc.compile()
        _CACHED[key] = nc
    return _CACHED[key]


def kernel(x, x_min, x_max, x_true, _trace=False):
    """Full-input entry point: shards across 8 cores, returns full outputs."""
    from concourse.bass_utils import run_bass_kernel_spmd

    x = np.asarray(x, dtype=np.float32)
    x_true = np.asarray(x_true, dtype=np.float32)
    assert x.shape == (ROWS, N_FULL), x.shape

    nshard = N_FULL // N_CORES
    nc = _get_program(nshard, 2048)

    in_maps = []
    for i in range(N_CORES):
        sl = slice(i * nshard, (i + 1) * nshard)
        in_maps.append({
            "x": np.ascontiguousarray(x[:, sl]),
            "x_true": np.ascontiguousarray(x_true[sl]),
        })

    res = run_bass_kernel_spmd(nc, in_maps, list(range(N_CORES)),
                               trace=_trace)
    outs = res.results

    x_new = np.concatenate([outs[i]["out_x"] for i in range(N_CORES)], axis=1)
    x_min_out = np.concatenate([outs[i]["out_min"] for i in range(N_CORES)])
    x_max_out = np.concatenate([outs[i]["out_max"] for i in range(N_CORES)])
    x_true_out = np.concatenate([outs[i]["out_true"] for i in range(N_CORES)])

    kernel.last_exec_time_ns = res.exec_time_ns
    return x_new, x_min_out, x_max_out, x_true_out


kernel.last_exec_time_ns = None


# revision 13
# speedup vs baseline: 3.3475x; 3.3475x over previous
"""Trainium2 Bass kernel for nn_AbstractReLU (interval-bound ReLU relaxation).

Computation (per column j of x[1026, N]):
  row 0 = center, rows 1..1024 = symbol coefs, row 1025 = noise.
  abs_sum = sum_{r>=1} |x[r,j]|;  lo = x0-abs_sum; hi = x0+abs_sum
  crossing (lo<0<hi):  mult = hi/(hi-lo), addend = 0.5*hi*(1-mult)
  positive (lo>=0,hi>0): mult = 1, addend = 0;  else mult = 0, addend = 0
  out row 0   = mult*x0 + addend
  out rows mid= mult*x
  out row 1025= mult*x_last + addend        (mult >= 0 so |mult| == mult)
  abs_sum2 = mult*(abs_sum - |x_last|) + |out_last|
  x_min_out = out0 - abs_sum2 ; x_max_out = out0 + abs_sum2
  x_true_out = relu(x_true)

Sharding: feature dim (axis 1 of x) split evenly across 8 cores; all
reductions are over the row axis which stays local -> no collectives.

Kernel layout per core (shard width NC):
  - rows on partitions: 9 row-tiles (8x128 + 1x2); column groups of G cols.
  - |x| on ScalarE, partition-reduce via ones-vector matmuls on TensorE
    accumulating into PSUM [1,512] banks.
  - per-column scalar chain on [32, NC/128] slab slices on VectorE.
  - mult broadcast to 128 partitions via K=1 matmul; 9 in-place DVE
    multiplies; stores via gpsimd DMA.
"""
import sys

for _p in ("/opt/trn_rl_repo",):
    if _p not in sys.path:
        sys.path.insert(0, _p)

import numpy as np
from contextlib import ExitStack

import concourse.bass as bass
import concourse.bacc as bacc
import concourse.tile as tile
from concourse import mybir

ROWS = 1026
N_FULL = 65536
N_CORES = 8

OP = mybir.AluOpType
DT = mybir.dt.float32
AF = mybir.ActivationFunctionType


def row_tiles():
    """(row_start, n_rows) for the 9 row tiles: 8x128 + 1x2."""
    ts = [(k * 128, 128) for k in range(8)]
    ts.append((1024, 2))
    return ts


def build_kernel(nc: bass.Bass, n_cols: int, group: int, reps: int = 1):
    """Emit the kernel program for one core's shard of width n_cols."""
    assert n_cols % 128 == 0 and n_cols % group == 0 and group % 512 == 0
    W = n_cols // 128          # slab free width (cols per partition)
    NG = n_cols // group       # number of column groups
    PPG = 128 // NG            # slab partitions per group
    assert PPG * W == group and PPG % 32 == 0 or NG == 1
    SUB = group // 512         # 512-col sub-blocks per group (PSUM bank size)
    PPS = 512 // W             # slab partitions per sub-block

    x_in = nc.dram_tensor("x", [ROWS, n_cols], DT, kind="ExternalInput").ap()
    xtrue_in = nc.dram_tensor("x_true", [n_cols], DT, kind="ExternalInput").ap()
    x_out = nc.dram_tensor("out_x", [ROWS, n_cols], DT, kind="ExternalOutput").ap()
    min_out = nc.dram_tensor("out_min", [n_cols], DT, kind="ExternalOutput").ap()
    max_out = nc.dram_tensor("out_max", [n_cols], DT, kind="ExternalOutput").ap()
    true_out = nc.dram_tensor("out_true", [n_cols], DT, kind="ExternalOutput").ap()

    with tile.TileContext(nc) as tc, ExitStack() as ctx:
        xtp = ctx.enter_context(tc.tile_pool(name="xt", bufs=2))
        abp = ctx.enter_context(tc.tile_pool(name="ab", bufs=2))
        srp = ctx.enter_context(tc.tile_pool(name="sr", bufs=2))
        mrp = srp
        slb = ctx.enter_context(tc.tile_pool(name="slb", bufs=1))
        onp = ctx.enter_context(tc.tile_pool(name="on", bufs=1))
        psa = ctx.enter_context(tc.tile_pool(name="psa", bufs=4, space="PSUM"))
        psb = ctx.enter_context(tc.tile_pool(name="psb", bufs=1, space="PSUM"))

        V, A = nc.vector, nc.scalar

        ones_col = onp.tile([128, 1], DT)     # lhsT for partition reduce
        ones_row = onp.tile([1, 128], DT)     # lhsT for broadcast
        V.memset(ones_col[:], 1.0)
        V.memset(ones_row[:], 1.0)

        # Persistent [128, W] slabs: column j <-> (partition j//W, free j%W).
        def slab(name):
            return slb.tile([128, W], DT, name=name)

        sall = slab("sall")   # sum |x| over ALL rows
        x0s = slab("x0s")
        xls = slab("xls")
        s1 = slab("s1")
        lo = slab("lo")
        hi = slab("hi")
        hip = slab("hip")
        maskp = slab("maskp")
        den = slab("den")
        rec = slab("rec")
        coef = slab("coef")
        mult = slab("mult")
        adde = slab("adde")
        new0 = slab("new0")
        newl = slab("newl")
        t1 = slab("t1")
        t2 = slab("t2")
        s2v = slab("s2v")
        minv = slab("minv")
        maxv = slab("maxv")

        for rep in range(reps):
          for g in range(NG):
            c0 = g * group
            cs = slice(c0, c0 + group)
            # ---- load rows 0..1023 as one DMA into a packed tile, plus
            # the 2-row tail tile.
            ta = xtp.tile([128, 8 * group], DT, tag="xta", name=f"xta_g{g}")
            nc.sync.dma_start(
                out=ta[:].rearrange("p (k c) -> p k c", k=8),
                in_=x_in[0:1024, cs].rearrange("(k p) c -> p k c", k=8))
            t8 = xtp.tile([2, group], DT, tag="xt8", name=f"xt8_g{g}")
            nc.sync.dma_start(out=t8[:], in_=x_in[1024:1026, cs])

            def kslice(k, s=None):
                if s is None:
                    return ta[:, k * group:(k + 1) * group]
                b0 = k * group + s * 512
                return ta[:, b0:b0 + 512]

            # ---- |x| and partition-reduce into PSUM accs (one per 512 cols)
            accs = [psa.tile([1, 512], DT, tag="acc", name=f"acc{s}_g{g}")
                    for s in range(SUB)]
            for k in range(9):
                nr = 2 if k == 8 else 128
                src = t8[:] if k == 8 else kslice(k)
                ab = abp.tile([nr, group], DT, tag="ab", name=f"ab{k}_g{g}")
                A.activation(ab[:], src, AF.Abs)
                for s in range(SUB):
                    nc.tensor.matmul(
                        accs[s][:],
                        ones_col[0:nr, :],
                        ab[:, s * 512:(s + 1) * 512],
                        start=(k == 0),
                        stop=(k == 8),
                    )

            # ---- move per-column sums into slab layout
            srow = srp.tile([1, group], DT, tag="rowbuf", name=f"sr_g{g}")
            for s in range(SUB):
                A.activation(srow[0:1, s * 512:(s + 1) * 512], accs[s][:],
                             AF.Copy)
            p0 = g * PPG
            gs = (slice(p0, p0 + PPG), slice(0, W))
            nc.gpsimd.dma_start(out=sall[gs], in_=srow[:])

            # row 0 and row 1025 into slab layout
            nc.gpsimd.dma_start(out=x0s[gs], in_=ta[0:1, 0:group])
            nc.gpsimd.dma_start(out=xls[gs], in_=t8[1:2, :])

            # ---- per-column scalar chain on [PPG, W] slices
            A.activation(t1[gs], x0s[gs], AF.Abs)                      # |x0|
            V.tensor_sub(s1[gs], sall[gs], t1[gs])                     # sum rows>=1
            V.tensor_sub(lo[gs], x0s[gs], s1[gs])
            V.tensor_add(hi[gs], x0s[gs], s1[gs])
            V.tensor_scalar(hip[gs], hi[gs], 0.0, None, OP.is_gt)
            V.tensor_scalar(t2[gs], lo[gs], 0.0, None, OP.is_lt)
            V.tensor_mul(maskp[gs], hip[gs], t2[gs])
            V.tensor_sub(den[gs], hi[gs], lo[gs])
            V.tensor_scalar(den[gs], den[gs], 1e-20, None, OP.max)
            V.reciprocal(rec[gs], den[gs])
            V.tensor_mul(coef[gs], hi[gs], rec[gs])
            V.tensor_scalar(coef[gs], coef[gs], 1.0, 0.0, OP.min, OP.max)
            V.tensor_mul(t1[gs], maskp[gs], coef[gs])
            V.tensor_sub(t2[gs], hip[gs], maskp[gs])
            V.tensor_add(mult[gs], t1[gs], t2[gs])
            V.tensor_scalar(t1[gs], coef[gs], -0.5, 0.5, OP.mult, OP.add)
            V.tensor_mul(t1[gs], t1[gs], hi[gs])                       # bias
            V.tensor_mul(adde[gs], maskp[gs], t1[gs])                  # addend
            V.tensor_mul(t1[gs], mult[gs], x0s[gs])
            V.tensor_add(new0[gs], t1[gs], adde[gs])
            V.tensor_mul(t1[gs], mult[gs], xls[gs])
            V.tensor_add(newl[gs], t1[gs], adde[gs])
            A.activation(t1[gs], xls[gs], AF.Abs)                      # |xl|
            V.tensor_sub(t2[gs], s1[gs], t1[gs])                       # mid sum
            V.tensor_mul(t2[gs], mult[gs], t2[gs])
            A.activation(t1[gs], newl[gs], AF.Abs)                     # |newl|
            V.tensor_add(s2v[gs], t2[gs], t1[gs])
            V.tensor_sub(minv[gs], new0[gs], s2v[gs])
            V.tensor_add(maxv[gs], new0[gs], s2v[gs])

            # ---- broadcast mult across 128 partitions via K=1 matmul
            mrow = mrp.tile([1, group], DT, tag="rowbuf", name=f"mr_g{g}")
            nc.gpsimd.dma_start(out=mrow[:], in_=mult[gs])
            bc = psb.tile([128, group], DT, tag="bc", name=f"bc_g{g}")
            for s in range(SUB):
                nc.tensor.matmul(
                    bc[:, s * 512:(s + 1) * 512],
                    ones_row[:],
                    mrow[0:1, s * 512:(s + 1) * 512],
                    start=True,
                    stop=True,
                )

            # ---- scale rows in place and store
            for k in range(8):
                V.tensor_mul(kslice(k), kslice(k), bc[:])
            V.tensor_mul(t8[:], t8[:], bc[0:2, :])
            nc.sync.dma_start(out=x_out[1:128, cs], in_=ta[1:128, 0:group])
            nc.sync.dma_start(
                out=x_out[128:1024, cs].rearrange("(k p) c -> p k c", k=7),
                in_=ta[:, group:8 * group].rearrange("p (k c) -> p k c", k=7))
            nc.sync.dma_start(out=x_out[1024:1025, cs], in_=t8[0:1, :])

        # ---- x_true relu
        xt_t = onp.tile([128, W], DT, name="xt_t")
        nc.sync.dma_start(out=xt_t[:], in_=xtrue_in[:])
        V.tensor_scalar(xt_t[:], xt_t[:], 0.0, None, OP.max)
        nc.gpsimd.dma_start(out=true_out[:], in_=xt_t[:])

        # ---- final row/vector outputs from slabs
        nc.gpsimd.dma_start(out=x_out[0:1, :], in_=new0[:])
        nc.gpsimd.dma_start(out=x_out[1025:1026, :], in_=newl[:])
        nc.gpsimd.dma_start(out=min_out[:], in_=minv[:])
        nc.gpsimd.dma_start(out=max_out[:], in_=maxv[:])

    return nc


_CACHED = {}


def _get_program(n_cols: int, group: int):
    key = (n_cols, group)
    if key not in _CACHED:
        nc = bacc.Bacc("TRN2", target_bir_lowering=False, debug=False,
                       num_devices=N_CORES)
        build_kernel(nc, n_cols, group)
        nc.compile()
        _CACHED[key] = nc
    return _CACHED[key]


def kernel(x, x_min, x_max, x_true, _trace=False):
    """Full-input entry point: shards across 8 cores, returns full outputs."""
    from concourse.bass_utils import run_bass_kernel_spmd

    x = np.asarray(x, dtype=np.float32)
    x_true = np.asarray(x_true, dtype=np.float32)
    assert x.shape == (ROWS, N_FULL), x.shape

    nshard = N_FULL // N_CORES
    nc = _get_program(nshard, 2048)

    in_maps = []
    for i in range(N_CORES):
        sl = slice(i * nshard, (i + 1) * nshard)
        in_maps.append({
            "x": np.ascontiguousarray(x[:, sl]),
            "x_true": np.ascontiguousarray(x_true[sl]),
        })

    res = run_bass_kernel_spmd(nc, in_maps, list(range(N_CORES)),
                               trace=_trace)
    outs = res.results

    x_new = np.concatenate([outs[i]["out_x"] for i in range(N_CORES)], axis=1)
    x_min_out = np.concatenate([outs[i]["out_min"] for i in range(N_CORES)])
    x_max_out = np.concatenate([outs[i]["out_max"] for i in range(N_CORES)])
    x_true_out = np.concatenate([outs[i]["out_true"] for i in range(N_CORES)])

    kernel.last_exec_time_ns = res.exec_time_ns
    return x_new, x_min_out, x_max_out, x_true_out


kernel.last_exec_time_ns = None


# revision 28
# speedup vs baseline: 14.4785x; 4.3252x over previous
"""Trainium2 Bass kernel for nn_AbstractReLU (interval-bound ReLU relaxation).

Computation (per column j of x[1026, N]):
  row 0 = center, rows 1..1024 = symbol coefs, row 1025 = noise.
  abs_sum = sum_{r>=1} |x[r,j]|;  lo = x0-abs_sum; hi = x0+abs_sum
  crossing (lo<0<hi):  mult = hi/(hi-lo), addend = 0.5*hi*(1-mult)
  positive (lo>=0,hi>0): mult = 1, addend = 0;  else mult = 0, addend = 0
  out row 0   = mult*x0 + addend
  out rows mid= mult*x
  out row 1025= mult*x_last + addend        (mult >= 0 so |mult| == mult)
  abs_sum2 = mult*(abs_sum - |x_last|) + |out_last|
  x_min_out = out0 - abs_sum2 ; x_max_out = out0 + abs_sum2
  x_true_out = relu(x_true)

Sharding: feature dim (axis 1 of x) split evenly across 8 cores; all
reductions are over the row axis which stays local -> no collectives.

Kernel layout per core (shard width NC):
  - rows on partitions: 9 row-tiles (8x128 + 1x2); column groups of G cols.
  - |x| on ScalarE, partition-reduce via ones-vector matmuls on TensorE
    accumulating into PSUM [1,512] banks.
  - per-column scalar chain on [32, NC/128] slab slices on VectorE.
  - mult broadcast to 128 partitions via K=1 matmul; 9 in-place DVE
    multiplies; stores via gpsimd DMA.
"""
import sys

for _p in ("/opt/trn_rl_repo",):
    if _p not in sys.path:
        sys.path.insert(0, _p)

import numpy as np
from contextlib import ExitStack

import concourse.bass as bass
import concourse.bacc as bacc
import concourse.tile as tile
from concourse import mybir

ROWS = 1026
N_FULL = 65536
N_CORES = 8

OP = mybir.AluOpType
DT = mybir.dt.float32
AF = mybir.ActivationFunctionType


def row_tiles():
    """(row_start, n_rows) for the 9 row tiles: 8x128 + 1x2."""
    ts = [(k * 128, 128) for k in range(8)]
    ts.append((1024, 2))
    return ts


BUILD_MARKS = []


def build_kernel(nc: bass.Bass, n_cols: int, group: int = 1024, reps: int = 1):
    """Emit the kernel program for one core's shard of width n_cols."""
    BUILD_MARKS.clear()

    def _mark(label):
        BUILD_MARKS.append((label, int(nc.get_next_instruction_name()[2:])))
    assert group == 1024 and n_cols % 4096 == 0
    SW = 32                    # slab free width; 32 partitions per group
    NSET = n_cols // 4096      # slab sets (128 partitions x SW cols each)
    NG = n_cols // group       # number of column groups
    SUB = group // 512         # 512-col sub-blocks per group (PSUM bank)
    WF = n_cols // 128

    x_in = nc.dram_tensor("x", [ROWS, n_cols], DT, kind="ExternalInput").ap()
    xtrue_in = nc.dram_tensor("x_true", [n_cols], DT, kind="ExternalInput").ap()
    x_out = nc.dram_tensor("out_x", [ROWS, n_cols], DT, kind="ExternalOutput").ap()
    min_out = nc.dram_tensor("out_min", [n_cols], DT, kind="ExternalOutput").ap()
    max_out = nc.dram_tensor("out_max", [n_cols], DT, kind="ExternalOutput").ap()
    true_out = nc.dram_tensor("out_true", [n_cols], DT, kind="ExternalOutput").ap()

    with tile.TileContext(nc) as tc, ExitStack() as ctx:
        xqp = ctx.enter_context(tc.tile_pool(name="xq", bufs=16))
        x8p = ctx.enter_context(tc.tile_pool(name="x8", bufs=2))
        abp = ctx.enter_context(tc.tile_pool(name="ab", bufs=2))
        csp = ctx.enter_context(tc.tile_pool(name="cs", bufs=3))
        srp = ctx.enter_context(tc.tile_pool(name="sr", bufs=2))
        slb = ctx.enter_context(tc.tile_pool(name="slb", bufs=2))
        onp = ctx.enter_context(tc.tile_pool(name="on", bufs=1))
        psa = ctx.enter_context(tc.tile_pool(name="psa", bufs=2, space="PSUM"))
        psb = ctx.enter_context(tc.tile_pool(name="psb", bufs=2, space="PSUM"))

        V, A, G = nc.vector, nc.scalar, nc.gpsimd

        ones_col = onp.tile([128, 1], DT)     # lhsT for partition reduce
        ones_row = onp.tile([1, 128], DT)     # lhsT for broadcast
        V.memset(ones_col[:], 1.0)
        V.memset(ones_row[:], 1.0)

        # Per-group [32, SW] stat tiles: local col j <-> (p j//SW, j%SW).
        names = ("sall x0s xls s1 lo hi hip maskp den rec coef mult adde "
                 "new0 newl t1 t2 s2v minv maxv").split()

        def chain_tiles(g):
            return {nm: slb.tile([32, SW], DT, tag=nm, name=f"{nm}_g{g}")
                    for nm in names}

        gstate = {}

        def stage_a(g):
            _mark(f"a{g}")
            c0 = g * group
            cs = slice(c0, c0 + group)
            S = chain_tiles(g)
            gs = (slice(0, 32), slice(0, SW))

            # ---- loads: one DMA per quarter (2 row-blocks) + 2-row tail
            xq = []
            for q in range(4):
                t = xqp.tile([128, 2 * group], DT, tag="xq",
                             name=f"xq{q}_g{g}")
                nc.sync.dma_start(
                    out=t[:].rearrange("p (k c) -> p k c", k=2),
                    in_=x_in[256 * q:256 * (q + 1), cs]
                        .rearrange("(k p) c -> p k c", k=2))
                xq.append(t)
            t8 = x8p.tile([2, group], DT, tag="xt8", name=f"xt8_g{g}")
            nc.sync.dma_start(out=t8[:], in_=x_in[1024:1026, cs])

            # ---- |x| per quarter, add-tree, one matmul per 512-col bank
            cs_t = []
            for q in range(4):
                ab = abp.tile([128, 2 * group], DT, tag="ab",
                              name=f"ab{q}_g{g}")
                A.activation(ab[:], xq[q][:], AF.Abs)
                cj = csp.tile([128, group], DT, tag="cs", name=f"cj{q}_g{g}")
                G.tensor_add(cj[:], ab[:, 0:group], ab[:, group:2 * group])
                cs_t.append(cj)
            d0 = csp.tile([128, group], DT, tag="ds", bufs=2, name=f"d0_g{g}")
            d1 = csp.tile([128, group], DT, tag="ds", bufs=2, name=f"d1_g{g}")
            V.tensor_add(d0[:], cs_t[0][:], cs_t[1][:])
            V.tensor_add(d1[:], cs_t[2][:], cs_t[3][:])
            ab8 = abp.tile([2, group], DT, tag="ab8", bufs=2,
                           name=f"ab8_g{g}")
            A.activation(ab8[:], t8[:], AF.Abs)
            G.tensor_add(d0[:], d0[:], d1[:])
            V.tensor_add(d0[0:2, :], d0[0:2, :], ab8[:])
            acc = psa.tile([1, group], DT, tag="acc", name=f"acc_g{g}")
            for s in range(SUB):
                nc.tensor.matmul(
                    acc[0:1, s * 512:(s + 1) * 512], ones_col[:],
                    d0[:, s * 512:(s + 1) * 512],
                    start=True, stop=True)

            # ---- per-column sums + rows 0/1025 into [32, SW] layout
            srow = srp.tile([1, group], DT, tag="rowb", bufs=3, name=f"sr_g{g}")
            A.activation(srow[:], acc[:], AF.Copy)
            A.dma_start(out=S["sall"][:], in_=srow[:])
            A.dma_start(out=S["x0s"][:], in_=xq[0][0:1, 0:group])
            A.dma_start(out=S["xls"][:], in_=t8[1:2, :])

            # ---- per-column scalar chain on [32, SW]
            A.activation(S["t1"][gs], S["x0s"][gs], AF.Abs)
            V.tensor_sub(S["s1"][gs], S["sall"][gs], S["t1"][gs])
            V.tensor_sub(S["lo"][gs], S["x0s"][gs], S["s1"][gs])
            V.tensor_add(S["hi"][gs], S["x0s"][gs], S["s1"][gs])
            V.tensor_scalar(S["hip"][gs], S["hi"][gs], 0.0, None, OP.is_gt)
            V.tensor_scalar(S["t2"][gs], S["lo"][gs], 0.0, None, OP.is_lt)
            V.tensor_mul(S["maskp"][gs], S["hip"][gs], S["t2"][gs])
            V.tensor_sub(S["den"][gs], S["hi"][gs], S["lo"][gs])
            V.tensor_scalar(S["den"][gs], S["den"][gs], 1e-20, None, OP.max)
            V.reciprocal(S["rec"][gs], S["den"][gs])
            V.tensor_mul(S["coef"][gs], S["hi"][gs], S["rec"][gs])
            V.tensor_scalar(S["coef"][gs], S["coef"][gs], 1.0, 0.0,
                            OP.min, OP.max)
            V.tensor_mul(S["t1"][gs], S["maskp"][gs], S["coef"][gs])
            V.tensor_sub(S["t2"][gs], S["hip"][gs], S["maskp"][gs])
            V.tensor_add(S["mult"][gs], S["t1"][gs], S["t2"][gs])
            V.tensor_scalar(S["t1"][gs], S["coef"][gs], -0.5, 0.5,
                            OP.mult, OP.add)
            V.tensor_mul(S["t1"][gs], S["t1"][gs], S["hi"][gs])
            V.tensor_mul(S["adde"][gs], S["maskp"][gs], S["t1"][gs])
            V.tensor_mul(S["t1"][gs], S["mult"][gs], S["x0s"][gs])
            V.tensor_add(S["new0"][gs], S["t1"][gs], S["adde"][gs])
            V.tensor_mul(S["t1"][gs], S["mult"][gs], S["xls"][gs])
            V.tensor_add(S["newl"][gs], S["t1"][gs], S["adde"][gs])
            A.activation(S["t1"][gs], S["xls"][gs], AF.Abs)
            V.tensor_sub(S["t2"][gs], S["s1"][gs], S["t1"][gs])
            V.tensor_mul(S["t2"][gs], S["mult"][gs], S["t2"][gs])
            A.activation(S["t1"][gs], S["newl"][gs], AF.Abs)
            V.tensor_add(S["s2v"][gs], S["t2"][gs], S["t1"][gs])
            V.tensor_sub(S["minv"][gs], S["new0"][gs], S["s2v"][gs])
            V.tensor_add(S["maxv"][gs], S["new0"][gs], S["s2v"][gs])

            # ---- broadcast mult across 128 partitions via K=1 matmul
            mrow = srp.tile([1, group], DT, tag="rowb", bufs=3, name=f"mr_g{g}")
            G.dma_start(out=mrow[:], in_=S["mult"][:])
            bc = psb.tile([128, group], DT, tag="bc", name=f"bc_g{g}")
            for s in range(SUB):
                nc.tensor.matmul(
                    bc[:, s * 512:(s + 1) * 512], ones_row[:],
                    mrow[0:1, s * 512:(s + 1) * 512],
                    start=True, stop=True)

            gstate[g] = (xq, t8, cs, S, bc)

        def stage_b(g):
            _mark(f"b{g}")
            xq, t8, cs, S, bc = gstate.pop(g)

            # ---- scale rows in place and store (per quarter)
            for q in range(4):
                V.tensor_mul(xq[q][:, 0:group], xq[q][:, 0:group], bc[:])
                V.tensor_mul(xq[q][:, group:2 * group],
                             xq[q][:, group:2 * group], bc[:])
                if q == 0:
                    nc.sync.dma_start(out=x_out[1:128, cs],
                                      in_=xq[0][1:128, 0:group])
                    nc.sync.dma_start(out=x_out[128:256, cs],
                                      in_=xq[0][:, group:2 * group])
                else:
                    nc.sync.dma_start(
                        out=x_out[256 * q:256 * (q + 1), cs]
                            .rearrange("(k p) c -> p k c", k=2),
                        in_=xq[q][:].rearrange("p (k c) -> p k c", k=2))
            V.tensor_mul(t8[:], t8[:], bc[0:2, :])
            nc.sync.dma_start(out=x_out[1024:1025, cs], in_=t8[0:1, :])
            # per-group row-0/row-1025/min/max outputs
            A.dma_start(out=x_out[0:1, cs], in_=S["new0"][:])
            A.dma_start(out=x_out[1025:1026, cs], in_=S["newl"][:])
            A.dma_start(out=min_out[cs], in_=S["minv"][:])
            A.dma_start(out=max_out[cs], in_=S["maxv"][:])

        for rep in range(reps):
            for g in range(NG):
                stage_a(g)
                if g >= 1:
                    stage_b(g - 1)
            stage_b(NG - 1)

        _mark("tail")
        # ---- x_true relu
        xt_t = onp.tile([128, WF], DT, name="xt_t")
        nc.sync.dma_start(out=xt_t[:], in_=xtrue_in[:])
        V.tensor_scalar(xt_t[:], xt_t[:], 0.0, None, OP.max)
        A.dma_start(out=true_out[:], in_=xt_t[:])

    return nc


_CACHED = {}


def _get_program(n_cols: int, group: int):
    key = (n_cols, group)
    if key not in _CACHED:
        nc = bacc.Bacc("TRN2", target_bir_lowering=False, debug=False,
                       num_devices=N_CORES)
        build_kernel(nc, n_cols, group)
        nc.compile()
        _CACHED[key] = nc
    return _CACHED[key]


def kernel(x, x_min, x_max, x_true, _trace=False):
    """Full-input entry point: shards across 8 cores, returns full outputs."""
    from concourse.bass_utils import run_bass_kernel_spmd

    x = np.asarray(x, dtype=np.float32)
    x_true = np.asarray(x_true, dtype=np.float32)
    assert x.shape == (ROWS, N_FULL), x.shape

    nshard = N_FULL // N_CORES
    nc = _get_program(nshard, 1024)

    in_maps = []
    for i in range(N_CORES):
        sl = slice(i * nshard, (i + 1) * nshard)
        in_maps.append({
            "x": np.ascontiguousarray(x[:, sl]),
            "x_true": np.ascontiguousarray(x_true[sl]),
        })

    res = run_bass_kernel_spmd(nc, in_maps, list(range(N_CORES)),
                               trace=_trace)
    outs = res.results

    x_new = np.concatenate([outs[i]["out_x"] for i in range(N_CORES)], axis=1)
    x_min_out = np.concatenate([outs[i]["out_min"] for i in range(N_CORES)])
    x_max_out = np.concatenate([outs[i]["out_max"] for i in range(N_CORES)])
    x_true_out = np.concatenate([outs[i]["out_true"] for i in range(N_CORES)])

    kernel.last_exec_time_ns = res.exec_time_ns
    return x_new, x_min_out, x_max_out, x_true_out


kernel.last_exec_time_ns = None
